# revision 18
# baseline (speedup 1.0000x reference)
"""BoeNet greedy BFS rollout — Trainium2 Bass kernel (8 NeuronCores).

Strategy (v2): fully data-parallel, no collectives.
  Each core takes 512 of the 4096 flattened positions.
  Host prep: embedding rows pre-gathered + transposed (pure layout),
  Wout converted to bf16 once (replicated to all cores).
  Phase A (f32r): h0 = embT@Wp+bp -> 3-level greedy tree rollout ->
  masked mean pool, kept transposed as bf16 [H, pos] tiles (= lhsT for
  phase B). Gate sigmoid(z)>0.5 computed as z > -c_d exactly.
  Aggregation: root+level1 contributions via DVE on materialized
  children; level-2 contributions via PE (Wcs = WcL+WcR) into PSUM.
  Phase B (bf16): logits[pos, :] = pooled @ Wout for the core's own
  positions x full vocab; Wout streamed from HBM in 8 vocab blocks of
  4000 cols (double-buffered), output written bf16, upcast on host.
  No AllGather: phase A feeds phase B directly (zero stall).
"""
import sys

for _p in ('/opt/trn_rl_repo', '/opt/pypackages'):
    if _p not in sys.path:
        sys.path.insert(0, _p)

import numpy as np

B, S, V, E, H = 8, 512, 32000, 512, 512
NPOS = B * S              # 4096 flattened positions
NCORES = 8
PC_POS = NPOS // NCORES   # 512 positions per core
VB = 16                   # vocab blocks per core
VBW = V // VB             # 2000 columns per block
VCW = 500                 # columns per psum tile
NVC = VBW // VCW          # 4 psum tiles per block
MAX_DEPTH = 3
DEPTH_EMBED_SCALE = 0.01
SIB_SCALE = 1.0 / np.sqrt(H)

_CACHE = {}


def _build():
    import concourse.bass as bass
    import concourse.bacc as bacc
    import concourse.tile as tile
    import concourse.mybir as mybir
    from contextlib import ExitStack

    F32 = mybir.dt.float32
    F32R = mybir.dt.float32r
    BF16 = mybir.dt.bfloat16
    AF = mybir.ActivationFunctionType
    OP = mybir.AluOpType

    nc = bacc.Bacc("TRN2", target_bir_lowering=False, debug=False,
                   num_devices=NCORES)

    hembt_d = nc.dram_tensor("hembt", [E, PC_POS], F32, kind="ExternalInput")
    wp_d = nc.dram_tensor("wp", [E, H], F32, kind="ExternalInput")
    wc_d = nc.dram_tensor("wc", [H, 2 * H], F32, kind="ExternalInput")
    wcs_d = nc.dram_tensor("wcs", [H, H], F32, kind="ExternalInput")
    wg_d = nc.dram_tensor("wg", [H, 1], F32, kind="ExternalInput")
    rows_d = nc.dram_tensor("rows", [5, H], F32, kind="ExternalInput")
    cols_d = nc.dram_tensor("cols", [128, 12], F32, kind="ExternalInput")
    thr_d = nc.dram_tensor("thr", [1, 4], F32, kind="ExternalInput")
    iden_d = nc.dram_tensor("iden", [128, 128], F32, kind="ExternalInput")
    wout_d = nc.dram_tensor("wout", [H, V], BF16, kind="ExternalInput")
    logits_d = nc.dram_tensor("logits", [PC_POS, V], BF16,
                              kind="ExternalOutput")

    R_BS = 3  # rows_d row index of biasL+biasR

    def cp(out_ap, in_ap):
        nc.scalar.activation(out_ap, in_ap, AF.Copy)

    with tile.TileContext(nc) as tc, ExitStack() as ctx:
        const = ctx.enter_context(tc.tile_pool(name="const", bufs=1))
        wres = ctx.enter_context(tc.tile_pool(name="wres", bufs=1))
        popool = ctx.enter_context(tc.tile_pool(name="popool", bufs=1))

        rows_sb = const.tile([1, 5 * H], F32R, tag="rows")
        ones_row = rows_sb[0:1, 4 * H:5 * H]
        thr_sb = const.tile([1, 4], F32, tag="thr")
        cols_sb = const.tile([128, 12], F32, tag="cols")
        wg_sb = const.tile([128, 4], F32R, tag="wg")
        identity = const.tile([128, 128], F32R, tag="ident")
        from concourse.library_config import standard as _std_lib
        nc.gpsimd.load_library(_std_lib)
        gwarm = const.tile([1, 16], F32, tag="gwarm")

        def load_consts():
            nc.sync.dma_start(rows_sb[:],
                              rows_d[:].rearrange("a b -> () (a b)").bitcast(F32R))
            nc.sync.dma_start(thr_sb[:], thr_d[:])
            nc.sync.dma_start(cols_sb[:], cols_d[:])
            for hc in range(4):
                nc.sync.dma_start(wg_sb[:, hc:hc + 1],
                                  wg_d[hc * 128:(hc + 1) * 128, :].bitcast(F32R))
            nc.sync.dma_start(identity[:], iden_d[:].bitcast(F32R))
            # pay the gpsimd IRAM library-load cost during startup DMA wait
            nc.gpsimd.memset(gwarm[:], 0.0)
            nc.gpsimd.tensor_copy(gwarm[:], gwarm[:])

        # pooled lhsT tiles for phase B (bf16, persistent, pc-split so
        # each LDWEIGHTS reads a whole tile from offset 0)
        po = [[popool.tile([128, 128], BF16, tag=f"po{jc}_{pc}",
                           name=f"po{jc}_{pc}") for pc in range(4)]
              for jc in range(4)]

        # ---------------- Phase A ----------------
        with ExitStack() as actx:
            npool = actx.enter_context(tc.tile_pool(name="npool", bufs=1))
            wcpool = actx.enter_context(tc.tile_pool(name="wcpool", bufs=1))
            chpool = actx.enter_context(tc.tile_pool(name="chpool", bufs=1))
            mpool = actx.enter_context(tc.tile_pool(name="mpool", bufs=8))
            ebpool = actx.enter_context(tc.tile_pool(name="ebpool", bufs=3))
            rpool = actx.enter_context(tc.tile_pool(name="rpool", bufs=1))
            vapool = actx.enter_context(tc.tile_pool(name="vapool", bufs=1))
            scr = actx.enter_context(tc.tile_pool(name="scr", bufs=3, space="PSUM"))
            espool = actx.enter_context(tc.tile_pool(name="espool", bufs=1, space="PSUM"))
            aggp = actx.enter_context(tc.tile_pool(name="aggp", bufs=4, space="PSUM"))

            # phase-A inputs (DMA priority order: earliest-needed first)
            hembT = []
            for ec in range(4):
                t = npool.tile([128, PC_POS], F32R, tag=f"he{ec}", name=f"he{ec}")
                nc.sync.dma_start(t[:], hembt_d[ec * 128:(ec + 1) * 128, :].bitcast(F32R))
                hembT.append(t)
            wp_sb = []
            for ec in range(4):
                t = npool.tile([128, 512], F32R, tag=f"wp{ec}", name=f"wp{ec}")
                nc.sync.dma_start(t[:], wp_d[ec * 128:(ec + 1) * 128, :].bitcast(F32R))
                wp_sb.append(t)
            load_consts()
            wc_sb = []
            for hc in range(4):
                t = wcpool.tile([128, 1024], F32R, tag=f"wc{hc}", name=f"wc{hc}")
                nc.sync.dma_start(t[:], wc_d[hc * 128:(hc + 1) * 128, :].bitcast(F32R))
                wc_sb.append(t)
            wcs_sb = []
            for hc in range(4):
                t = npool.tile([128, 512], F32R, tag=f"wcs{hc}", name=f"wcs{hc}")
                nc.sync.dma_start(t[:], wcs_d[hc * 128:(hc + 1) * 128, :].bitcast(F32R))
                wcs_sb.append(t)

            # phase-B streamed weights: first two vocab blocks prefetch now
            # (drain during phase A); rest stream via the ring in phase B.
            wo_tiles = {}

            def wo_load(vb):
                tl = []
                for hc in range(4):
                    t = wres.tile([128, VBW], BF16, tag=f"wo{hc}",
                                  name=f"wo{hc}_{vb}", bufs=2)
                    nc.sync.dma_start(
                        t[:], wout_d[hc * 128:(hc + 1) * 128,
                                     vb * VBW:(vb + 1) * VBW])
                    tl.append(t)
                wo_tiles[vb] = tl

            wo_load(0)
            wo_load(1)

            # h0 = embT@Wp + bp  (bp folded into psum->sbuf copy bias)
            h0_sb = []
            for hc in range(4):
                ps = scr.tile([128, 512], F32, tag="s", name="h0ps")
                for ec in range(4):
                    nc.tensor.matmul(ps[:], wp_sb[ec][:, hc * 128:(hc + 1) * 128],
                                     hembT[ec][:], start=(ec == 0), stop=(ec == 3))
                t = npool.tile([128, 512], F32R, tag=f"h0_{hc}", name=f"h0_{hc}")
                nc.scalar.activation(t[:], ps[:], AF.Identity,
                                     bias=cols_sb[:, hc:hc + 1])
                h0_sb.append(t)

            # agg accumulator in PSUM, initialized with the root (h0) term
            # via identity matmul (h0_sb already includes bp)
            agg_ps = []
            for jc in range(4):
                ap_ = aggp.tile([128, 512], F32, tag="agg", name=f"agg{jc}")
                nc.tensor.matmul(ap_[:], identity[:], h0_sb[jc][:],
                                 start=True, stop=False, skip_group_check=True)
                agg_ps.append(ap_)

            def gate(node, depth, parent_e):
                zp = scr.tile([1, 512], F32, tag="s", name="zp")
                for hc in range(4):
                    nc.tensor.matmul(zp[:], wg_sb[:, hc:hc + 1], node[hc][:],
                                     start=(hc == 0), stop=(hc == 3))
                e = rpool.tile([1, 512], F32R, tag="erow", name="erow", bufs=7)
                nc.vector.tensor_scalar(e[:], zp[:], thr_sb[0:1, depth:depth + 1],
                                        None, OP.is_gt)
                if parent_e is not None:
                    nc.vector.tensor_mul(e[:], e[:], parent_e[:])
                return e

            esum_bc = espool.tile([128, 512], F32, tag="esb", name="esum_bc")
            eb_calls = [0]

            def ebroadcast(e_row):
                ebp = scr.tile([128, 512], F32, tag="s", name="ebp")
                nc.tensor.matmul(ebp[:], ones_row[0:1, 0:128], e_row[:],
                                 start=True, stop=True)
                # accumulate the broadcast expands: esum_bc = sum_k ones (x) e_k
                nc.tensor.matmul(esum_bc[:], ones_row[0:1, 0:128], e_row[:],
                                 start=(eb_calls[0] == 0), stop=(eb_calls[0] == 6),
                                 skip_group_check=True)
                eb_calls[0] += 1
                eb = ebpool.tile([128, 512], F32R, tag="eb", name="eb")
                cp(eb[:], ebp[:])
                return eb

            # q = sum_k e_k * node_k  (masked-node accumulator; the child
            # contributions are linear: sum_k Wcs^T(e_k*node_k) = Wcs^T q)
            q_sb = [vapool.tile([128, 512], F32R, tag=f"q{hc}", name=f"q{hc}")
                    for hc in range(4)]
            q_first = [True]

            def q_accum(node, eb):
                if q_first[0]:
                    for hc in range(4):
                        eng = nc.gpsimd if hc >= 2 else nc.vector
                        eng.tensor_mul(q_sb[hc][:], node[hc][:], eb[:])
                    q_first[0] = False
                    return
                for hc in range(4):
                    meng = nc.gpsimd if hc >= 2 else nc.vector
                    aeng = nc.vector if hc >= 2 else nc.gpsimd
                    m = mpool.tile([128, 512], F32R, tag="mn", name=f"mn{hc}")
                    meng.tensor_mul(m[:], node[hc][:], eb[:])
                    aeng.tensor_add(q_sb[hc][:], q_sb[hc][:], m[:])

            def children(node, lvl, nbufs):
                out = []
                for side in (0, 1):
                    child = []
                    for jc2 in range(4):
                        jq = side * 4 + jc2
                        ps = scr.tile([128, 512], F32, tag="s", name="chps")
                        for hc in range(4):
                            nc.tensor.matmul(ps[:], wc_sb[hc][:, jq * 128:(jq + 1) * 128],
                                             node[hc][:], start=(hc == 0), stop=(hc == 3))
                        t = chpool.tile([128, 512], F32R, tag=f"ch{lvl}",
                                        name=f"ch{lvl}_{side}_{jc2}", bufs=nbufs)
                        nc.scalar.activation(
                            t[:], ps[:], AF.Identity,
                            bias=cols_sb[:, 4 + side * 4 + jc2: 5 + side * 4 + jc2])
                        child.append(t)
                    out.append(child)
                return out

            with nc.allow_low_precision(reason="f32r matmul inputs"):
                # level-1 children first (pure PE work), gates/DVE behind
                n10, n11 = children(h0_sb, 1, 8)
                e0 = gate(h0_sb, 0, None)
                q_accum(h0_sb, ebroadcast(e0))

                n20, n21 = children(n10, 2, 16)
                e10 = gate(n10, 1, e0)
                q_accum(n10, ebroadcast(e10))
                e20 = gate(n20, 2, e10)
                e21 = gate(n21, 2, e10)

                n22, n23 = children(n11, 2, 16)
                e11 = gate(n11, 1, e0)
                q_accum(n11, ebroadcast(e11))
                eb20 = ebroadcast(e20)
                eb21 = ebroadcast(e21)
                e22 = gate(n22, 2, e11)
                eb22 = ebroadcast(e22)
                e23 = gate(n23, 2, e11)
                eb23 = ebroadcast(e23)

                # esum row for the deferred child-bias term, and the
                # reciprocal-count broadcast 1/(1+2*esum) — DVE/scalar work
                # that overlaps the four level-2 contribution bursts below
                esum_row = rpool.tile([1, 512], F32R, tag="esr", name="esr", bufs=1)
                cp(esum_row[:], esum_bc[0:1, :])
                rbt = ebpool.tile([128, 512], F32, tag="rbt", name="rbt", bufs=1)
                nc.vector.tensor_scalar(rbt[:], esum_bc[:], 2.0, 1.0,
                                        OP.mult, OP.add)
                nc.vector.reciprocal(rbt[:], rbt[:])

                q_accum(n20, eb20)
                q_accum(n21, eb21)
                q_accum(n22, eb22)
                q_accum(n23, eb23)
                # single contraction of the masked-node sum into agg
                for jc in range(4):
                    for hc in range(4):
                        nc.tensor.matmul(agg_ps[jc][:],
                                         wcs_sb[hc][:, jc * 128:(jc + 1) * 128],
                                         q_sb[hc][:], start=False, stop=False,
                                         skip_group_check=True)
                for jc in range(4):
                    nc.tensor.matmul(agg_ps[jc][:],
                                     rows_sb[0:1, R_BS * H + jc * 128: R_BS * H + (jc + 1) * 128],
                                     esum_row[:], start=False, stop=True,
                                     skip_group_check=True)
                # pooled = agg_ps * (1/cnt), to bf16 lhsT tiles (pc-split,
                # pc-major so phase B's first group unblocks earliest)
                for pc in range(4):
                    for jc in range(4):
                        nc.vector.tensor_mul(
                            po[jc][pc][:], agg_ps[jc][:, pc * 128:(pc + 1) * 128],
                            rbt[:, pc * 128:(pc + 1) * 128])

        # ---------------- Phase B ----------------
        with ExitStack() as bctx, \
                nc.allow_low_precision(reason="bf16 matmul inputs"):
            stp = bctx.enter_context(tc.tile_pool(name="stp", bufs=2))
            mmp = bctx.enter_context(tc.tile_pool(name="mmp", bufs=8, space="PSUM"))

            for vb in range(VB):
                for pc in range(4):
                    pst = [mmp.tile([128, VCW], F32, tag="mm", name=f"mm{v}",
                                    bufs=8) for v in range(NVC)]
                    for hc in range(4):
                        for v in range(NVC):
                            nc.tensor.matmul(
                                pst[v][:],
                                po[hc][pc][:],
                                wo_tiles[vb][hc][:, v * VCW:(v + 1) * VCW],
                                start=(hc == 0), stop=(hc == 3))
                    stage = stp.tile([128, VBW], BF16, tag="stage", name="stage")
                    for v in range(NVC):
                        dst = stage[:, v * VCW:(v + 1) * VCW]
                        if v == 0 or v == 3:
                            nc.vector.tensor_copy(dst, pst[v][:])
                        else:
                            cp(dst, pst[v][:])
                    nc.scalar.dma_start(
                        logits_d[pc * 128:(pc + 1) * 128,
                                 vb * VBW:(vb + 1) * VBW],
                        stage[:])
                # stream in the block after next (ring bufs=2)
                if vb + 2 < VB:
                    wo_load(vb + 2)

    nc.compile()
    return nc


def _get_nc():
    if "nc" not in _CACHE:
        _CACHE["nc"] = _build()
    return _CACHE["nc"]


def _prep_inputs(tokens, emb, Wp, bp, Wc, bc, Wg, bg, dep, sib, Wout, bout):
    import ml_dtypes
    tokens = np.asarray(tokens).astype(np.int64).reshape(-1)
    emb = np.asarray(emb, dtype=np.float32)
    Wp = np.ascontiguousarray(np.asarray(Wp, dtype=np.float32))
    bp = np.asarray(bp, dtype=np.float32).reshape(-1)
    Wc = np.asarray(Wc, dtype=np.float32)
    bc = np.asarray(bc, dtype=np.float32).reshape(-1)
    Wg = np.ascontiguousarray(np.asarray(Wg, dtype=np.float32))
    bg = np.asarray(bg, dtype=np.float32).reshape(-1)
    dep = np.asarray(dep, dtype=np.float32)
    sib = np.asarray(sib, dtype=np.float32)
    Wout = np.asarray(Wout, dtype=np.float32)
    bout = np.asarray(bout, dtype=np.float32).reshape(-1)
    _CACHE["bout"] = bout.copy()

    wcs = np.ascontiguousarray(Wc[:, :H] + Wc[:, H:])
    biasL = bc[:H] + SIB_SCALE * sib[0]
    biasR = bc[H:] + SIB_SCALE * sib[1]
    rows = np.ascontiguousarray(
        np.stack([bp, biasL, biasR, biasL + biasR, np.ones(H, np.float32)]))
    cols = np.ascontiguousarray(np.concatenate(
        [bp.reshape(4, 128).T, biasL.reshape(4, 128).T, biasR.reshape(4, 128).T],
        axis=1).astype(np.float32))
    g = DEPTH_EMBED_SCALE * (dep[:MAX_DEPTH] @ Wg[:, 0]) + bg[0]
    thr = np.zeros((1, 4), np.float32)
    thr[0, :MAX_DEPTH] = -g

    wout_bf = np.ascontiguousarray(Wout.astype(ml_dtypes.bfloat16))
    iden = np.eye(128, dtype=np.float32)
    wc_c = np.ascontiguousarray(Wc)

    in_maps = []
    for c in range(NCORES):
        tk = tokens[c * PC_POS:(c + 1) * PC_POS]
        hembt = np.ascontiguousarray(emb[tk].T)  # [E, PC_POS] f32
        in_maps.append({
            "hembt": hembt, "wp": Wp, "wc": wc_c, "wcs": wcs, "wg": Wg,
            "rows": rows, "cols": cols, "thr": thr, "iden": iden,
            "wout": wout_bf,
        })
    return in_maps


def _post(res) -> np.ndarray:
    parts = [np.asarray(res.results[c]["logits"]) for c in range(NCORES)]
    logits = np.concatenate(parts, axis=0).astype(np.float32)
    bout = _CACHE.get("bout")
    if bout is not None and np.any(bout):
        logits += bout
    return logits.reshape(B, S, V)


def _enable_ldw_opt_once():
    # Flip walrus's --enable-ldw-opt for compiles issued from this process
    # (dedups back-to-back identical LDWEIGHTS; measured win, verified exact).
    import os
    if os.environ.get("NO_LDW_OPT"):
        return
    if _CACHE.get("ldw_patched"):
        return
    import concourse.bass_utils as bu
    _orig = bu.run_command

    def _patched(cmd, **kw):
        if isinstance(cmd, list):
            cmd = ["--enable-ldw-opt=true" if c == "--enable-ldw-opt=false" else c
                   for c in cmd]
        return _orig(cmd, **kw)

    bu.run_command = _patched
    _CACHE["ldw_patched"] = True


def kernel(**inputs) -> np.ndarray:
    from concourse.bass_utils import run_bass_kernel_spmd
    _enable_ldw_opt_once()
    nc = _get_nc()
    in_maps = _prep_inputs(**inputs)
    res = run_bass_kernel_spmd(nc, in_maps, list(range(NCORES)))
    return _post(res)
